# revision 57
# baseline (speedup 1.0000x reference)
"""GBST Trainium2 kernel (nn_GBST_42434276884940), v4.

Self-contained: takes FULL inputs, shards batch over 8 NeuronCores
(2 rows/core), runs a Bass/Tile kernel per core, gathers full output.
HW ~52-54us/exec (baseline indirect-DMA/fp32 version: 143us), rel err
~1e-2 vs the 2e-2 gate.

Embedding lookup is an on-device one-hot matmul (vocab=256): per
128-position chunk, ohT[v,p] = (id[p]==v) via DVE is_equal (bf16, 2x
mode), then X = ohT0.T@T4[0:128] + ohT1.T@T4[128:256] (+ the period-4
positional encoding, which depends only on p%4 inside a chunk: added
either by a K=4 matmul or fused into the psum->SBUF copy as a DVE
tensor-add, whichever engine has slack). The score projection rides
along as table column 256. All matmuls are bf16; psum stays f32; the
output is written bf16 and upcast on the host.

Pipelining structure (from TimelineSim analysis):
- one packed const DMA + idxb in 5 slices on the ACT HWDGE ring, so
  the first compares start ~2us in.
- chunk PAIRS share a 2-bank psum tile -> one psum->SBUF copy per two
  chunks, alternating DVE (fused pe-add) / ACT.
- psum pools are SCOPED: gather phase = 3x2 xg banks + scT; released
  before the build/apply phase opens a double-buffered 3-bank G tile
  (gm|gl|gh merged, every matmul slice in-bank) + 2 out2 pair banks.
- softmax + calibration + C build run in slot ranges 0:13 / 13:24 /
  24:48 so DVE work overlaps PE score matmuls and builds start early;
  0.5*C[m=0] is folded into the gm accumulation via smats[0]=0.5*I so
  the Gsb assemble is three plain ACT copies.
- apply matmuls write out2 pairs; output DMAs merged 26 -> 10.
"""

import os
import sys

import numpy as np

if "/opt/trn_rl_repo" not in sys.path:
    sys.path.insert(0, "/opt/trn_rl_repo")

import ml_dtypes

import concourse.bass as bass
import concourse.tile as tile
from concourse import bacc, mybir
from concourse.bass_utils import run_bass_kernel_spmd

F32 = mybir.dt.float32
BF16 = mybir.dt.bfloat16

MAX_BLOCK = 4
EMBED = 256
VOCAB = 256
BATCH = 16
SEQ = 3072
NCORES = 8
BLOC = BATCH // NCORES           # 2
NPOS = BLOC * SEQ                # 6144
NCHUNK = NPOS // 128             # 48
NGROUP = 4
GSZ = NCHUNK // NGROUP           # 12
ELEM = EMBED + 1                 # 257: embedding + score column

SLOTS = [0, 3, 6, 9, 2, 5, 8, 11, 1, 4, 7, 10]   # slot s -> tau_l
SLOT_OF = {t: s for s, t in enumerate(SLOTS)}
CLASS_TAUL = [[0, 3, 6, 9], [2, 5, 8, 11], [1, 4, 7, 10]]

NPBF = ml_dtypes.bfloat16


# ---------------------------------------------------------------- host consts

def _sinusoidal_pe(max_len, d):
    pos = np.arange(max_len, dtype=np.float32)[:, None]
    div = np.exp(np.arange(0, d, 2, dtype=np.float32) * (-np.log(10000.0) / d))
    pe = np.zeros((max_len, d), dtype=np.float32)
    pe[:, 0::2] = np.sin(pos * div)
    pe[:, 1::2] = np.cos(pos * div)
    return pe


def build_cpack(embed_table, w_score):
    """Packed small constants, one DMA: [t4 (2x257) | viota (2x128) |
    pe4 (rows 0-3, 257) | phase4 (rows 0-3, 128)] per partition."""
    table = np.asarray(embed_table, dtype=np.float32)
    w = np.asarray(w_score, dtype=np.float32).reshape(EMBED)
    pe = _sinusoidal_pe(MAX_BLOCK, EMBED)
    cp = np.zeros((128, CPACK_W), dtype=np.float32)
    for kb in range(2):
        rows = table[128 * kb:128 * (kb + 1)]
        cp[:, kb * ELEM:kb * ELEM + EMBED] = rows
        cp[:, kb * ELEM + EMBED] = rows @ w
    v = np.arange(128, dtype=np.float32)
    for kb in range(2):
        cp[:, 2 * ELEM + kb * 128:2 * ELEM + (kb + 1) * 128] = \
            (v + 128 * kb)[:, None]
    cp[0:4, 770:770 + EMBED] = pe
    cp[0:4, 770 + EMBED] = pe @ w
    p = np.arange(128)
    cp[0:4, 1027:1155] = (np.arange(4)[:, None] == p[None, :] % 4)
    # peT: the positional addend as a full [128, ELEM] tile (row p%4),
    # consumed by the fused psum->X add on DVE-copied chunk pairs
    cp[:, 1155:1155 + EMBED] = pe[p % 4]
    cp[:, 1155 + EMBED] = (pe @ w)[p % 4]
    return cp.astype(NPBF)


def phi_of_taul(tau_l):
    return (2 * tau_l) % 3


def build_smats():
    k = np.arange(128)
    mats = np.zeros((12, 128, 128), dtype=np.float32)
    mats[0] = 0.5 * np.eye(128, dtype=np.float32)
    mats[1] = 0.25 * (k[:, None] // 2 == k[None, :] // 2)
    mats[2] = 0.125 * (k[:, None] // 4 == k[None, :] // 4)
    for phi in range(3):
        mats[3 + phi] = (1 / 6) * ((k[:, None] + phi) // 3 == (k[None, :] + phi) // 3)
        mats[6 + phi] = (1 / 6) * ((128 + k[:, None] + phi) // 3 == (k[None, :] + phi) // 3)
        mats[9 + phi] = (1 / 6) * ((k[:, None] - 128 + phi) // 3 == (k[None, :] + phi) // 3)
    return mats.astype(NPBF)


def build_m2w():
    p = np.arange(128)
    j = np.arange(64)
    m2 = (j[None, :] == p[:, None] // 2).astype(np.float32)   # [128, 64]
    return np.ascontiguousarray(
        np.broadcast_to(m2[:, :, None], (128, 64, NCHUNK))).astype(NPBF)


def build_idxb(input_ids):
    """Per-core bf16 [128, 48, 128]: idxb[q, c, p] = token id at position
    128c + p, replicated down the partition (vocab-compare) axis."""
    ids = np.asarray(input_ids).astype(np.int64)
    out = []
    for core in range(NCORES):
        lin = ids[core * BLOC:(core + 1) * BLOC].reshape(NPOS)
        grid = lin.reshape(NCHUNK, 128).astype(np.float32)
        out.append(np.ascontiguousarray(
            np.broadcast_to(grid[None], (128, NCHUNK, 128))).astype(NPBF))
    return out


# ---------------------------------------------------------------- device prog

def _view(ap, offset_delta, free_dims):
    """Manual free-dim view of an AP: keep partition dim, replace free dims.
    free_dims: list of [stride_elems, count]."""
    return bass.AP(tensor=ap.tensor, offset=ap.offset + offset_delta,
                   ap=[list(ap.ap[0])] + [list(d) for d in free_dims])


CPACK_W = 2 * ELEM + 2 * 128 + ELEM + 128 + ELEM    # 1412 (incl peT)


def emit_program(nc, nrep=1, phases=("gather", "scores", "tail")):
    idxb_d = nc.dram_tensor("idxb", [128, NCHUNK, 128], BF16, kind="ExternalInput")
    cpack_d = nc.dram_tensor("cpack", [128, CPACK_W], BF16, kind="ExternalInput")
    smats_d = nc.dram_tensor("smats", [12, 128, 128], BF16, kind="ExternalInput")
    m2w_d = nc.dram_tensor("m2w", [128, 64, NCHUNK], BF16, kind="ExternalInput")
    out_d = nc.dram_tensor("out", [NPOS // 2, EMBED], BF16, kind="ExternalOutput")

    with tile.TileContext(nc) as tc:
        with (
            tc.tile_pool(name="consts", bufs=1) as consts,
            tc.tile_pool(name="big", bufs=1) as big,
            tc.tile_pool(name="ohp", bufs=6) as ohp,
            tc.tile_pool(name="sm", bufs=1) as sm,
            tc.tile_pool(name="outsb", bufs=2) as outsb_pool,
        ):
            # ---- constants to SBUF, ordered by first use ----
            # one packed DMA for the small gather constants; idxb slices go
            # on the ACT HWDGE ring so they overlap the SP-ring loads.
            cpack_sb = consts.tile([128, CPACK_W], BF16, tag="cpack")
            nc.sync.dma_start(cpack_sb[:], cpack_d.ap()[:, :])
            cp_ap = cpack_sb[:]
            t4_v = [_view(cp_ap, kb * ELEM, [[1, ELEM]]) for kb in range(2)]
            viota_v = _view(cp_ap, 2 * ELEM, [[128, 2], [1, 128]])
            pe4_v = cpack_sb[0:4, 2 * ELEM + 256:2 * ELEM + 256 + ELEM]
            phase4_v = cpack_sb[0:4, 3 * ELEM + 256:3 * ELEM + 384]
            peT_off = 3 * ELEM + 384
            idxb_sb = consts.tile([128, NCHUNK, 128], BF16, tag="idxb")
            for a, b in ((0, 6), (6, 12), (12, 24), (24, 36), (36, 48)):
                nc.scalar.dma_start(idxb_sb[:, a:b, :],
                                    idxb_d.ap()[:, a:b, :])
            smats_sb = consts.tile([128, 12, 128], BF16, tag="smats")
            nc.sync.dma_start(
                smats_sb[:],
                bass.AP(tensor=smats_d, offset=0,
                        ap=[[128, 128], [128 * 128, 12], [1, 128]]))
            m2w_sb = consts.tile([128, 64, NCHUNK], BF16, tag="m2w")
            nc.sync.dma_start(m2w_sb[:], m2w_d.ap()[:, :, :])

            # ---- persistent big tensors ----
            X = big.tile([128, NCHUNK, ELEM], BF16, tag="X")
            C = big.tile([128, 4, 64, NCHUNK], BF16, tag="C")
            Gsb = big.tile([128, NCHUNK, 128], BF16, tag="Gsb")
            d2 = big.tile([128, NCHUNK], BF16, tag="d2")     # slot-ordered
            c4m = big.tile([128, 4, NCHUNK], BF16, tag="c4m")  # m-major

            def mmat(out_ap, mi, rhs_ap, start, stop):
                nc.tensor.matmul(out=out_ap, lhsT=smats_sb[:, mi, :],
                                 rhs=rhs_ap, start=start, stop=stop,
                                 skip_group_check=True)

            def emit_gather(g, xg_ps):
                g0 = g * GSZ
                for h in range(2):      # half-group compare batches
                    ohT = ohp.tile([128, 6, 2, 128], BF16, tag="ohT")
                    h0 = g0 + 6 * h
                    nc.vector.tensor_tensor(
                        out=ohT[:],
                        in0=idxb_sb[:, h0:h0 + 6, :].unsqueeze(2)
                            .to_broadcast([128, 6, 2, 128]),
                        in1=viota_v.unsqueeze(1)
                            .to_broadcast([128, 6, 2, 128]),
                        op=mybir.AluOpType.is_equal)
                    for cp in range(3):     # chunk pairs: one psum tile each
                        xg = xg_ps.tile([128, 1024], F32, tag="xg")
                        on_dve = (h * 3 + cp) % 2 == 0
                        for half in range(2):
                            c = 2 * cp + half
                            xv = _view(xg[:], 512 * half, [[1, ELEM]])
                            nc.tensor.matmul(out=xv, lhsT=ohT[:, c, 0, :],
                                             rhs=t4_v[0], start=True,
                                             stop=False, skip_group_check=True)
                            nc.tensor.matmul(out=xv, lhsT=ohT[:, c, 1, :],
                                             rhs=t4_v[1], start=False,
                                             stop=on_dve,
                                             skip_group_check=True)
                            if not on_dve:
                                # ACT copies can't add; keep the pe-add mm
                                nc.tensor.matmul(out=xv, lhsT=phase4_v,
                                                 rhs=pe4_v, start=False,
                                                 stop=True,
                                                 skip_group_check=True)
                        cc = h0 + 2 * cp
                        xpair = _view(xg[:], 0, [[512, 2], [1, ELEM]])
                        if on_dve:
                            # fused psum->SBUF copy + positional-encoding add
                            nc.vector.tensor_tensor(
                                out=X[:, cc:cc + 2, :], in0=xpair,
                                in1=_view(cp_ap, peT_off,
                                          [[0, 2], [1, ELEM]]),
                                op=mybir.AluOpType.add)
                        else:
                            nc.scalar.copy(X[:, cc:cc + 2, :], xpair)
                # d2 = 2*d, written in slot order: class c block <- tau_l
                # stride-3 run (score path feeds half-scaled matrices)
                for c in range(3):
                    t0 = CLASS_TAUL[c][0]
                    nc.scalar.mul(
                        d2[:, g0 + 4 * c:g0 + 4 * c + 4].unsqueeze(2),
                        X[:, g0 + t0:g0 + t0 + 10:3, EMBED:EMBED + 1], 2.0)

            def emit_scores_m013(gpair, scT):
                # m in {0,1,3} uses one smats matrix with contiguous
                # d2/scT ranges -> one N=24 matmul per group pair
                s0 = 24 * gpair
                for m, mi in ((0, 0), (1, 1), (3, 2)):
                    mmat(scT[:, m, s0:s0 + 24], mi, d2[:, s0:s0 + 24],
                         gpair == 0 and m == 0, False)

            def emit_scores(g, scT):
                g0 = g * GSZ          # chunk/slot offset (same space)
                S = 12 * g            # slot column offset in scT

                def sc_mm(m, o0, on, mi, src0, srcn, start, stop):
                    mmat(scT[:, m, S + o0:S + o0 + on], mi,
                         d2[:, src0:src0 + srcn], start, stop)

                for c in range(3):
                    phi = phi_of_taul(CLASS_TAUL[c][0])
                    sc_mm(2, 4 * c, 4, 3 + phi, g0 + 4 * c, 4, False, False)
                up_sc = [(0, 0, 4, 8), (1, 4, 3, 1), (2, 8, 4, 4)]
                dn_sc = [(0, 1, 3, 4), (1, 4, 4, 8), (2, 8, 4, 0)]
                for plan, base in ((up_sc, 6), (dn_sc, 9)):
                    for c, o0, on, s0 in plan:
                        phi = phi_of_taul(CLASS_TAUL[c][0])
                        sc_mm(2, o0, on, base + phi, g0 + s0, on, False, False)
                if g % 2 == 0:   # up-fix: slot 7 (tau_l 11) <- next grp slot 0
                    sc_mm(2, 7, 1, 6 + phi_of_taul(11), (g + 1) * GSZ, 1,
                          False, False)
                else:            # dn-fix: slot 0 <- prev group slot 7
                    sc_mm(2, 0, 1, 9 + phi_of_taul(0), g0 - GSZ + 7, 1,
                          False, g == 3)

            def emit_softmax_half(src_ap, src_w, src_s0, hh, s0, HC):
                # softmax + tiny self-attn calibration over a slot range;
                # src read through a [128, slot, m] transposed view.
                # src_ap is the scT psum tile OR its SBUF staging copy
                # (src_w = its slot width; src_s0 relative to it); s0 is
                # the GLOBAL slot offset for the c4m/C outputs.
                scT_t = _view(src_ap, src_s0, [[1, HC], [src_w, 4]])
                ex = sm.tile([128, HC, 4], BF16, tag=f"ex{hh}")
                nc.scalar.activation(out=ex[:], in_=scT_t,
                                     func=mybir.ActivationFunctionType.Exp)
                Z = sm.tile([128, HC], F32, tag=f"Z{hh}")
                nc.vector.tensor_reduce(out=Z[:], in_=ex[:],
                                        axis=mybir.AxisListType.X,
                                        op=mybir.AluOpType.add)
                rz = sm.tile([128, HC], F32, tag=f"rz{hh}")
                nc.vector.reciprocal(out=rz[:], in_=Z[:])
                r = sm.tile([128, HC, 4], BF16, tag=f"r{hh}")
                nc.vector.tensor_tensor(
                    out=r[:], in0=ex[:],
                    in1=rz[:].unsqueeze(2).to_broadcast([128, HC, 4]),
                    op=mybir.AluOpType.mult)
                P = sm.tile([128, HC, 4, 4], BF16, tag=f"P{hh}")
                nc.vector.tensor_tensor(
                    out=P[:],
                    in0=r[:].unsqueeze(3).to_broadcast([128, HC, 4, 4]),
                    in1=r[:].unsqueeze(2).to_broadcast([128, HC, 4, 4]),
                    op=mybir.AluOpType.mult)
                E = sm.tile([128, HC, 4, 4], BF16, tag=f"E{hh}")
                nc.scalar.activation(out=E[:], in_=P[:],
                                     func=mybir.ActivationFunctionType.Exp)
                D = sm.tile([128, HC, 4], F32, tag=f"D{hh}")
                nc.vector.tensor_reduce(out=D[:], in_=E[:],
                                        axis=mybir.AxisListType.X,
                                        op=mybir.AluOpType.add)
                EN = sm.tile([128, HC, 4, 4], BF16, tag=f"EN{hh}")
                nc.vector.tensor_tensor(
                    out=EN[:], in0=E[:],
                    in1=r[:].unsqueeze(2).to_broadcast([128, HC, 4, 4]),
                    op=mybir.AluOpType.mult)
                Nn = sm.tile([128, HC, 4], F32, tag=f"Nn{hh}")
                nc.vector.tensor_reduce(out=Nn[:], in_=EN[:],
                                        axis=mybir.AxisListType.X,
                                        op=mybir.AluOpType.add)
                rD = sm.tile([128, HC, 4], F32, tag=f"rD{hh}")
                nc.vector.reciprocal(out=rD[:], in_=D[:])
                # c4 written m-major through a transposed view of c4m
                c4m_t = _view(c4m[:], s0, [[1, HC], [NCHUNK, 4]])
                nc.vector.tensor_tensor(out=c4m_t, in0=Nn[:], in1=rD[:],
                                        op=mybir.AluOpType.mult)
                # C[p, m, j, slot] = c4m[p, m, slot] * m2w[p, j, slot];
                # the first quarter is split per-m (build-consumption order
                # m1,m3,m2,m0) so builds(0) starts earlier.
                if hh == 0:
                    for m in (1, 3, 2, 0):
                        C_h = _view(C[:], m * 64 * NCHUNK + s0,
                                    [[NCHUNK, 64], [1, HC]])
                        nc.vector.tensor_tensor(
                            out=C_h,
                            in0=_view(c4m[:], m * NCHUNK + s0,
                                      [[0, 64], [1, HC]]),
                            in1=_view(m2w_sb[:], s0, [[NCHUNK, 64], [1, HC]]),
                            op=mybir.AluOpType.mult)
                else:
                    C_h = _view(C[:], s0,
                                [[64 * NCHUNK, 4], [NCHUNK, 64], [1, HC]])
                    nc.vector.tensor_tensor(
                        out=C_h,
                        in0=_view(c4m[:], s0, [[NCHUNK, 4], [0, 64], [1, HC]]),
                        in1=_view(m2w_sb[:], s0,
                                  [[0, 4], [NCHUNK, 64], [1, HC]]),
                        op=mybir.AluOpType.mult)

            # C views for build-matmul rhs: slot-major [slots, j-range]
            def c_rhs(m, j0, jn, s0, sn):
                return _view(C[:], m * 64 * NCHUNK + j0 * NCHUNK + s0,
                             [[1, sn], [NCHUNK, jn]])

            def emit_builds(g, G):
                g0 = g * GSZ
                gbase = G[:]
                # flat G layout (f32 elems): gm [12 slots x 64] at 0,
                # gl [12 x 32] at 768, gh [12 x 32] at 1152.
                def gm_v(s0, sn):
                    return _view(gbase, 64 * s0, [[64, sn], [1, 64]])

                def gl_v(s0, sn):
                    return _view(gbase, 768 + 32 * s0, [[32, sn], [1, 32]])

                def gh_v(s0, sn):
                    return _view(gbase, 1152 + 32 * s0, [[32, sn], [1, 32]])

                # m2/m4 pooled contributions (banks 0 and 1 started here)
                mmat(gm_v(0, 8), 1, c_rhs(1, 0, 64, g0, 8), True, False)
                mmat(gm_v(8, 4), 1, c_rhs(1, 0, 64, g0 + 8, 4), True, False)
                mmat(gm_v(0, 8), 2, c_rhs(3, 0, 64, g0, 8), False, False)
                mmat(gm_v(8, 4), 2, c_rhs(3, 0, 64, g0 + 8, 4), False, False)
                for c in range(3):
                    phi = phi_of_taul(CLASS_TAUL[c][0])
                    mmat(gm_v(4 * c, 4), 3 + phi,
                         c_rhs(2, 0, 64, g0 + 4 * c, 4), False, False)
                # smats[0] = 0.5*eye: accumulate 0.5*C[m=0] on the PE so the
                # Gsb assemble is a plain ACT copy instead of a DVE stt
                mmat(gm_v(0, 8), 0, c_rhs(0, 0, 64, g0, 8), False, True)
                mmat(gm_v(8, 4), 0, c_rhs(0, 0, 64, g0 + 8, 4), False, True)
                # gl: dn contributions; gl[8:12] is bank 2's first writer
                dn_plan = [(2, 8, 4, 0), (0, 1, 3, 4), (1, 4, 4, 8)]
                for i, (c, o0, on, s0) in enumerate(dn_plan):
                    phi = phi_of_taul(CLASS_TAUL[c][0])
                    mmat(gl_v(o0, on), 9 + phi,
                         c_rhs(2, 32, 32, g0 + s0, on), i == 0, True)
                if g % 2 == 1:
                    mmat(gl_v(0, 1), 9 + phi_of_taul(0),
                         c_rhs(2, 32, 32, g0 - 12 + 7, 1), False, True)
                up_plan = [(0, 0, 4, 8), (1, 4, 3, 1), (2, 8, 4, 4)]
                for c, o0, on, s0 in up_plan:
                    phi = phi_of_taul(CLASS_TAUL[c][0])
                    mmat(gh_v(o0, on), 6 + phi,
                         c_rhs(2, 0, 32, g0 + s0, on), False, True)
                if g % 2 == 0:
                    mmat(gh_v(7, 1), 6 + phi_of_taul(11),
                         c_rhs(2, 0, 32, g0 + 12, 1), False, True)
                    nc.vector.memset(gl_v(0, 1), 0.0)
                else:
                    nc.vector.memset(gh_v(7, 1), 0.0)
                # assemble Gsb (bf16): gm already includes 0.5*C[m=0]
                nc.scalar.copy(Gsb[:, g0:g0 + GSZ, 32:96], gm_v(0, GSZ))
                nc.scalar.copy(Gsb[:, g0:g0 + GSZ, 0:32], gl_v(0, GSZ))
                nc.scalar.copy(Gsb[:, g0:g0 + GSZ, 96:128], gh_v(0, GSZ))

            def gsb_idx(row, tt):
                g = 2 * row + tt // GSZ
                return g * GSZ + SLOT_OF[tt % GSZ]

            def emit_big(row, pair_list, osb, out2_ps):
                # pair_list: list of (ot0, n) with n in {1, 2}; each pair
                # shares one psum bank (odd ot's matmuls never set start —
                # the even ot's first matmul poisons the whole bank).
                for pi, (ot0, np_) in enumerate(pair_list):
                    out2 = out2_ps.tile([128, 2, EMBED], F32, tag="out2")
                    for i in range(np_):
                        ot = ot0 + i
                        tt_e = 2 * ot
                        first = (i == 0)
                        if tt_e < 24:
                            nc.tensor.matmul(
                                out=out2[:, i, :],
                                lhsT=Gsb[:, gsb_idx(row, tt_e), :],
                                rhs=X[:, 24 * row + tt_e, 0:EMBED],
                                start=first, stop=False,
                                skip_group_check=True)
                        if tt_e - 1 >= 0:
                            nc.tensor.matmul(
                                out=out2[0:64, i, :],
                                lhsT=Gsb[:, gsb_idx(row, tt_e - 1), 64:128],
                                rhs=X[:, 24 * row + tt_e - 1, 0:EMBED],
                                start=(first and tt_e >= 24), stop=True,
                                skip_group_check=True)
                        if tt_e + 1 < 24:
                            nc.tensor.matmul(
                                out=out2[64:128, i, :],
                                lhsT=Gsb[:, gsb_idx(row, tt_e + 1), 0:64],
                                rhs=X[:, 24 * row + tt_e + 1, 0:EMBED],
                                start=False, stop=True,
                                skip_group_check=True)
                    # copies start at partition 0 (engine partition-base
                    # rule); ot==0 copies garbage rows 0:32 too (DMA skips).
                    c1 = 32 if ot0 == 12 else 128
                    nc.scalar.copy(osb[0:c1, ot0:ot0 + np_, :],
                                   out2[0:c1, 0:np_, :])

            PAIRS_A = [(0, 2), (2, 2), (4, 2)]
            PAIRS_B1 = [(6, 2), (8, 2)]
            PAIRS_B2 = [(10, 2), (12, 1)]

            def emit_row_dmas(row, osb, part):
                rb = row * (SEQ // 2)
                if part == 0:
                    nc.sync.dma_start(
                        bass.AP(tensor=out_d, offset=rb * EMBED,
                                ap=[[EMBED, 96], [1, EMBED]]),
                        osb[32:128, 0, :])
                elif part == 1:
                    nc.sync.dma_start(
                        bass.AP(tensor=out_d, offset=(rb + 96) * EMBED,
                                ap=[[EMBED, 128], [128 * EMBED, 5],
                                    [1, EMBED]]),
                        _view(osb[:], 1 * EMBED, [[EMBED, 5], [1, EMBED]]))
                else:
                    nc.sync.dma_start(
                        bass.AP(tensor=out_d, offset=(rb + 736) * EMBED,
                                ap=[[EMBED, 128], [128 * EMBED, 4],
                                    [1, EMBED]]),
                        _view(osb[:], 6 * EMBED, [[EMBED, 4], [1, EMBED]]))
                    nc.sync.dma_start(
                        bass.AP(tensor=out_d, offset=(rb + 1248) * EMBED,
                                ap=[[EMBED, 128], [128 * EMBED, 2],
                                    [1, EMBED]]),
                        _view(osb[:], 10 * EMBED, [[EMBED, 2], [1, EMBED]]))
                    nc.sync.dma_start(
                        bass.AP(tensor=out_d, offset=(rb + 1504) * EMBED,
                                ap=[[EMBED, 32], [1, EMBED]]),
                        osb[0:32, 12, :])

            # ---- staged pipeline ----
            # Scope 2 is flattened: row-1 builds are interleaved into the
            # row-0 apply stream so PE fills the softmax-H2/assemble waits.
            for _rep in range(nrep):
                with (
                    tc.tile_pool(name="xg_ps", bufs=3, space="PSUM") as xg_ps,
                    tc.tile_pool(name="scT_ps", bufs=1, space="PSUM") as scT_ps,
                ):
                    scT = scT_ps.tile([128, 4, NCHUNK], F32, tag="scT")
                    emit_gather(0, xg_ps)
                    emit_gather(1, xg_ps)
                    if "scores" in phases:
                        emit_scores_m013(0, scT)
                        emit_scores(0, scT)
                        emit_scores(1, scT)
                    emit_gather(2, xg_ps)
                    emit_gather(3, xg_ps)
                    if "scores" in phases:
                        emit_softmax_half(scT[:], NCHUNK, 0, 0, 0, 13)
                        emit_softmax_half(scT[:], NCHUNK, 13, 1, 13, 11)
                        emit_scores_m013(1, scT)
                        emit_scores(2, scT)
                        emit_scores(3, scT)
                        # stage scT's upper half to SBUF so the scT psum
                        # bank releases right after the last score matmul
                        # (scope-2's G pool overlaps its zone and would
                        # otherwise wait for the softmax-H2 exp reads)
                        scS2 = sm.tile([128, 4, 24], F32, tag="scS2")
                        nc.vector.tensor_copy(out=scS2[:],
                                              in_=scT[:, :, 24:48])
                        emit_softmax_half(scS2[:], 24, 0, 2, 24, 13)
                        emit_softmax_half(scS2[:], 24, 13, 3, 37, 11)
                if "tail" not in phases:
                    continue
                with (
                    tc.tile_pool(name="g_ps", bufs=2, space="PSUM") as g_ps,
                    tc.tile_pool(name="out2_ps", bufs=2, space="PSUM") as out2_ps,
                ):
                    osb0 = outsb_pool.tile([128, 13, EMBED], BF16, tag="osb")
                    G_a = g_ps.tile([128, 1536], F32, tag="G")
                    emit_builds(0, G_a)
                    G_b = g_ps.tile([128, 1536], F32, tag="G")
                    emit_builds(1, G_b)
                    emit_big(0, PAIRS_A, osb0, out2_ps)
                    G_c = g_ps.tile([128, 1536], F32, tag="G")
                    emit_builds(2, G_c)
                    emit_row_dmas(0, osb0, 0)
                    emit_big(0, PAIRS_B1, osb0, out2_ps)
                    G_d = g_ps.tile([128, 1536], F32, tag="G")
                    emit_builds(3, G_d)
                    emit_row_dmas(0, osb0, 1)
                    emit_big(0, PAIRS_B2, osb0, out2_ps)
                    emit_row_dmas(0, osb0, 2)
                    osb1 = outsb_pool.tile([128, 13, EMBED], BF16, tag="osb")
                    emit_big(1, PAIRS_A, osb1, out2_ps)
                    emit_row_dmas(1, osb1, 0)
                    emit_big(1, PAIRS_B1, osb1, out2_ps)
                    emit_row_dmas(1, osb1, 1)
                    emit_big(1, PAIRS_B2, osb1, out2_ps)
                    emit_row_dmas(1, osb1, 2)

    return nc


_CACHE = {}


def _get_nc(nrep=1):
    phases = tuple(os.environ.get(
        "GBST_PHASES", "gather,scores,tail").split(","))
    key = f"nc{nrep}-{phases}"
    if key not in _CACHE:
        nc = bacc.Bacc("TRN2", target_bir_lowering=False, debug=False)
        emit_program(nc, nrep=nrep, phases=phases)
        nc.compile()
        _CACHE[key] = nc
    return _CACHE[key]


def prepare_in_maps(input_ids, embed_table, w_score, b_score=None):
    # b_score only shifts all 4 scores equally -> softmax-invariant; unused.
    cpack = build_cpack(embed_table, w_score)
    smats = build_smats()
    m2w = build_m2w()
    idxbs = build_idxb(input_ids)
    return [{"idxb": idxbs[core], "cpack": cpack,
             "smats": smats, "m2w": m2w} for core in range(NCORES)]


def assemble_out(results):
    outs = [results[c]["out"].astype(np.float32).reshape(BLOC, SEQ // 2, EMBED)
            for c in range(NCORES)]
    return np.concatenate(outs, axis=0)


def kernel(input_ids, embed_table, w_score, b_score):
    in_maps = prepare_in_maps(input_ids, embed_table, w_score, b_score)
    res = run_bass_kernel_spmd(_get_nc(), in_maps,
                               core_ids=list(range(NCORES)))
    return assemble_out(res.results)
